# revision 1
# baseline (speedup 1.0000x reference)
"""Cross-attention Trainium2 kernel, batch-parallel across 8 NeuronCores.

Per core: one batch element. LN(x) -> qT via transposed projection,
LN(ctx) -> kT / v, transposed-layout attention (keys on partitions, no
P-transposes), softmax normalization via reciprocal + PE ones-broadcast,
out projection + final LN in row space. All matmuls bf16 with f32 PSUM
accumulation. LN scale/bias and the num_heads**-0.5 factor are folded
into the weights on the host.
"""
import numpy as np
import ml_dtypes

import concourse.bass as bass
from concourse import bacc
import concourse.mybir as mybir
import concourse.tile as tile
from concourse.bass_utils import run_bass_kernel_spmd
from concourse.masks import make_identity

BF = mybir.dt.bfloat16
F32 = mybir.dt.float32
NPBF = ml_dtypes.bfloat16

B, N_FULL, M, DIM = 8, 4096, 256, 1024
H, D = 16, 64
INNER = H * D
EPS = 1e-6
SCALE = H ** -0.5

_cache = {}


def _build(n_rows, apply_mask, trivial_lno):
    nchunks = n_rows // 512
    nc = bacc.Bacc(None, target_bir_lowering=False)
    x_d = nc.dram_tensor("x", [n_rows, DIM], BF, kind="ExternalInput")
    ctx_d = nc.dram_tensor("ctx", [M, DIM], BF, kind="ExternalInput")
    wq_d = nc.dram_tensor("wq", [DIM, INNER], BF, kind="ExternalInput")
    wk_d = nc.dram_tensor("wk", [DIM, INNER], BF, kind="ExternalInput")
    wv_d = nc.dram_tensor("wv", [DIM, INNER], BF, kind="ExternalInput")
    wo_d = nc.dram_tensor("wo", [INNER, DIM], BF, kind="ExternalInput")
    wmean_d = nc.dram_tensor("wmean", [INNER, 1], BF, kind="ExternalInput")  # -Wo@1/DIM
    nullkblk_d = nc.dram_tensor("nullkblk", [128, 8, 16], BF, kind="ExternalInput")
    nullv1_d = nc.dram_tensor("nullv1", [1, 65], BF, kind="ExternalInput")
    maskcol_d = nc.dram_tensor("maskcol", [128, 2], BF, kind="ExternalInput")
    lnos_d = nc.dram_tensor("lnos", [1, DIM], F32, kind="ExternalInput")
    lnob_d = nc.dram_tensor("lnob", [1, DIM], F32, kind="ExternalInput")
    out_d = nc.dram_tensor("out", [n_rows, DIM], F32, kind="ExternalOutput")

    with tile.TileContext(nc) as tc:
        with tc.tile_pool(name="const", bufs=1) as cst, \
             tc.tile_pool(name="sbw", bufs=1) as sbw, \
             tc.tile_pool(name="sbx", bufs=3) as sbx, \
             tc.tile_pool(name="sbq", bufs=2) as sbq, \
             tc.tile_pool(name="sba", bufs=2) as sba, \
             tc.tile_pool(name="sbo", bufs=2) as sbo, \
             tc.tile_pool(name="pproj", bufs=2, space="PSUM") as pproj, \
             tc.tile_pool(name="psim", bufs=3, space="PSUM") as psim, \
             tc.tile_pool(name="pout", bufs=2, space="PSUM") as pout, \
             tc.tile_pool(name="paux", bufs=1, space="PSUM") as paux, \
             tc.tile_pool(name="drp", bufs=3, space="DRAM") as drp:

            ident = cst.tile([128, 128], BF, tag="ident")
            make_identity(nc, ident)
            epst = cst.tile([128, 1], F32, tag="epst")
            nc.vector.memset(epst, EPS)
            ones64 = cst.tile([1, 64], BF, tag="ones64")
            nc.vector.memset(ones64, 1.0)
            nullv1 = cst.tile([1, 65], BF, tag="nullv1")
            nc.sync.dma_start(out=nullv1, in_=nullv1_d[:, :])
            nullkblk = cst.tile([128, 8, 16], BF, tag="nullkblk")
            nc.sync.dma_start(out=nullkblk, in_=nullkblk_d[:, :, :])
            wmean = cst.tile([128, 8, 1], BF, tag="wmean")
            nc.sync.dma_start(out=wmean, in_=wmean_d.rearrange("(j p) o -> p j o", p=128))
            if apply_mask:
                maskcol = cst.tile([128, 2], BF, tag="maskcol")
                nc.sync.dma_start(out=maskcol, in_=maskcol_d[:, :])
            if not trivial_lno:
                lnos = cst.tile([128, DIM], F32, tag="lnos")
                lnob = cst.tile([128, DIM], F32, tag="lnob")
                nc.sync.dma_start(out=lnos, in_=bass.AP(
                    tensor=lnos_d, offset=0, ap=[[0, 128], [1, DIM]]))
                nc.sync.dma_start(out=lnob, in_=bass.AP(
                    tensor=lnob_d, offset=0, ap=[[0, 128], [1, DIM]]))

            # weights: [128, j, ...] partition-tiled over contraction dim
            wq = sbw.tile([128, 8, INNER], BF, tag="wq")
            wk = sbw.tile([128, 8, INNER], BF, tag="wk")
            wv = sbw.tile([128, 8, INNER], BF, tag="wv")
            wo = sbw.tile([128, 8, DIM], BF, tag="wo")
            for j in range(8):
                nc.sync.dma_start(out=wq[:, j], in_=wq_d.rearrange("(j p) i -> p j i", p=128)[:, j])
                nc.sync.dma_start(out=wk[:, j], in_=wk_d.rearrange("(j p) i -> p j i", p=128)[:, j])
                nc.sync.dma_start(out=wv[:, j], in_=wv_d.rearrange("(j p) i -> p j i", p=128)[:, j])
                nc.sync.dma_start(out=wo[:, j], in_=wo_d.rearrange("(j p) i -> p j i", p=128)[:, j])

            def layernorm_rows(dst_bf, src_tile, tmp_pool):
                """LN rows of [128, DIM] src -> bf16 dst. Returns nothing."""
                stats = tmp_pool.tile([128, 2, 6], F32, tag="stats")
                nc.vector.bn_stats(stats[:, 0, :], src_tile[:, 0:512])
                nc.vector.bn_stats(stats[:, 1, :], src_tile[:, 512:1024])
                mv = tmp_pool.tile([128, 2], F32, tag="mv")
                nc.vector.bn_aggr(mv, stats)
                rstd = tmp_pool.tile([128, 1], F32, tag="rstd")
                nc.scalar.activation(rstd, mv[:, 1:2], mybir.ActivationFunctionType.Ln,
                                     bias=epst, scale=1.0)
                nc.scalar.activation(rstd, rstd, mybir.ActivationFunctionType.Exp, scale=-0.5)
                nc.vector.tensor_scalar(out=dst_bf, in0=src_tile,
                                        scalar1=mv[:, 0:1], scalar2=rstd,
                                        op0=mybir.AluOpType.subtract,
                                        op1=mybir.AluOpType.mult)

            # ---------------- context phase ----------------
            cnT = sbw.tile([128, 8, 256], BF, tag="cnT")
            for mm in range(2):
                ctile = sbx.tile([128, DIM], BF, tag="ctile")
                nc.sync.dma_start(out=ctile, in_=ctx_d[128 * mm:128 * (mm + 1), :])
                cn = sbx.tile([128, DIM], BF, tag="cn")
                layernorm_rows(cn, ctile, sbx)
                for g in range(2):  # transpose 8 blocks -> 2 psum tiles
                    ptr = pproj.tile([128, 512], BF, tag="proj")
                    for b4 in range(4):
                        jj = g * 4 + b4
                        nc.tensor.transpose(ptr[:, 128 * b4:128 * (b4 + 1)],
                                            cn[:, 128 * jj:128 * (jj + 1)], ident)
                    nc.vector.tensor_copy(
                        cnT[:, g * 4:(g + 1) * 4, 128 * mm:128 * (mm + 1)],
                        ptr.rearrange("p (a b) -> p a b", a=4))

            kT = sbw.tile([128, 8, 256], BF, tag="kT")
            for i in range(8):
                pk = pproj.tile([128, 512], F32, tag="proj")
                for j in range(8):
                    nc.tensor.matmul(pk[:, 0:256], wk[:, j, 128 * i:128 * (i + 1)],
                                     cnT[:, j, :], start=(j == 0), stop=(j == 7))
                nc.vector.tensor_copy(kT[:, i, :], pk[:, 0:256])

            v_sb = sbw.tile([128, 2, 16, 65], BF, tag="v_sb")
            for mm in range(2):
                for nh in range(2):
                    pv = pproj.tile([128, 512], F32, tag="proj")
                    for j in range(8):
                        nc.tensor.matmul(pv, cnT[:, j, 128 * mm:128 * (mm + 1)],
                                         wv[:, j, 512 * nh:512 * (nh + 1)],
                                         start=(j == 0), stop=(j == 7))
                    nc.vector.tensor_copy(
                        v_sb[:, mm, 8 * nh:8 * (nh + 1), 0:64],
                        pv.rearrange("p (h d) -> p h d", h=8))
                nc.vector.memset(v_sb[:, mm, :, 64:65], 1.0)

            # ---------------- main loop over row chunks ----------------
            for c in range(nchunks):
                xnT = sbq.tile([128, 8, 512], BF, tag="xnT")
                for r in range(4):
                    xbf = sbx.tile([128, DIM], BF, tag="xbf")
                    nc.sync.dma_start(out=xbf, in_=x_d[c * 512 + 128 * r: c * 512 + 128 * (r + 1), :])
                    xn = sbx.tile([128, DIM], BF, tag="xn")
                    layernorm_rows(xn, xbf, sbx)
                    for g in range(2):
                        ptr = pproj.tile([128, 512], BF, tag="proj")
                        for b4 in range(4):
                            jj = g * 4 + b4
                            nc.tensor.transpose(ptr[:, 128 * b4:128 * (b4 + 1)],
                                                xn[:, 128 * jj:128 * (jj + 1)], ident)
                        nc.vector.tensor_copy(
                            xnT[:, g * 4:(g + 1) * 4, 128 * r:128 * (r + 1)],
                            ptr.rearrange("p (a b) -> p a b", a=4))

                qT = sbq.tile([128, 8, 512], BF, tag="qT")
                for i in range(8):
                    pq = pproj.tile([128, 512], F32, tag="proj")
                    for j in range(8):
                        nc.tensor.matmul(pq, wq[:, j, 128 * i:128 * (i + 1)], xnT[:, j, :],
                                         start=(j == 0), stop=(j == 7))
                    nc.vector.tensor_copy(qT[:, i, :], pq)

                # null-key scores for all heads: [16, 512]
                pnull = pproj.tile([128, 512], F32, tag="proj")
                for j in range(8):
                    nc.tensor.matmul(pnull[0:16, :], nullkblk[:, j, :], qT[:, j, :],
                                     start=(j == 0), stop=(j == 7))
                enull16 = sba.tile([16, 512], BF, tag="enull16")
                nc.scalar.activation(enull16, pnull[0:16, :], mybir.ActivationFunctionType.Exp)
                enullf = sba.tile([1, 16, 512], BF, tag="enullf")
                nc.sync.dma_start(out=enullf, in_=enull16)

                outT = sbo.tile([128, 8, 512], BF, tag="outT")
                for h in range(H):
                    j, po = h // 2, 64 * (h % 2)
                    ps0 = psim.tile([128, 512], F32, tag="sim")
                    ps1 = psim.tile([128, 512], F32, tag="sim")
                    nc.tensor.matmul(ps0, kT[po:po + 64, j, 0:128], qT[po:po + 64, j, :],
                                     start=True, stop=True)
                    nc.tensor.matmul(ps1, kT[po:po + 64, j, 128:256], qT[po:po + 64, j, :],
                                     start=True, stop=True)
                    eT = sba.tile([128, 2, 512], BF, tag="eT")
                    nc.scalar.activation(eT[:, 0, :], ps0, mybir.ActivationFunctionType.Exp)
                    nc.scalar.activation(eT[:, 1, :], ps1, mybir.ActivationFunctionType.Exp)
                    if apply_mask:
                        nc.vector.tensor_scalar_mul(eT[:, 0, :], in0=eT[:, 0, :],
                                                    scalar1=maskcol[:, 0:1])
                        nc.vector.tensor_scalar_mul(eT[:, 1, :], in0=eT[:, 1, :],
                                                    scalar1=maskcol[:, 1:2])
                    po_ps = pout.tile([65, 512], F32, tag="out")
                    nc.tensor.matmul(po_ps, v_sb[:, 0, h, :], eT[:, 0, :], start=True, stop=False)
                    nc.tensor.matmul(po_ps, v_sb[:, 1, h, :], eT[:, 1, :], start=False, stop=False)
                    nc.tensor.matmul(po_ps, nullv1, enullf[0:1, h, :], start=False, stop=True)
                    rec = sba.tile([1, 512], F32, tag="rec")
                    nc.vector.reciprocal(rec, po_ps[64:65, :])
                    rec_d = drp.tile([1, 512], F32, tag="rec_d")
                    nc.sync.dma_start(out=rec_d[:, :], in_=rec)
                    recb = sba.tile([64, 512], F32, tag="recb")
                    nc.sync.dma_start(out=recb, in_=bass.AP(
                        tensor=rec_d.tensor, offset=rec_d.offset, ap=[[0, 64], [1, 512]]))
                    nc.vector.tensor_mul(outT[po:po + 64, j, :], po_ps[0:64, :], recb)

                # out projection + final LN (row space)
                for m in range(4):
                    pmean = pout.tile([128, 1], F32, tag="out")
                    for j in range(8):
                        nc.tensor.matmul(pmean, outT[:, j, 128 * m:128 * (m + 1)],
                                         wmean[:, j, :], start=(j == 0), stop=(j == 7))
                    negmu = sbx.tile([128, 1], F32, tag="negmu")
                    nc.vector.tensor_copy(negmu, pmean)
                    fins = []
                    sumsqs = []
                    for nh in range(2):
                        pf = pproj.tile([128, 512], F32, tag="proj")
                        for j in range(8):
                            nc.tensor.matmul(pf, outT[:, j, 128 * m:128 * (m + 1)],
                                             wo[:, j, 512 * nh:512 * (nh + 1)],
                                             start=(j == 0), stop=(j == 7))
                        junk = sbx.tile([128, 512], BF, tag="junk")
                        ssq = sbx.tile([128, 1], F32, tag=f"ssq{nh}")
                        nc.scalar.activation(junk, pf, mybir.ActivationFunctionType.Square,
                                             bias=negmu, scale=1.0, accum_out=ssq)
                        fins.append(pf)
                        sumsqs.append(ssq)
                    var = sbx.tile([128, 1], F32, tag="var")
                    nc.vector.tensor_add(var, sumsqs[0], sumsqs[1])
                    rstd_o = sbx.tile([128, 1], F32, tag="rstd_o")
                    nc.scalar.activation(rstd_o, var, mybir.ActivationFunctionType.Ln,
                                         bias=epst, scale=1.0 / DIM)
                    nc.scalar.activation(rstd_o, rstd_o, mybir.ActivationFunctionType.Exp,
                                         scale=-0.5)
                    orow = sbo.tile([128, DIM], F32, tag="orow")
                    for nh in range(2):
                        nc.vector.tensor_scalar(out=orow[:, 512 * nh:512 * (nh + 1)],
                                                in0=fins[nh], scalar1=negmu, scalar2=rstd_o,
                                                op0=mybir.AluOpType.add,
                                                op1=mybir.AluOpType.mult)
                    if not trivial_lno:
                        nc.vector.tensor_mul(orow, orow, lnos)
                        nc.vector.tensor_add(orow, orow, lnob)
                    nc.sync.dma_start(out=out_d[c * 512 + 128 * m: c * 512 + 128 * (m + 1), :],
                                      in_=orow)
    nc.compile()
    return nc


def _get_nc(n_rows, apply_mask, trivial_lno):
    key = (n_rows, apply_mask, trivial_lno)
    if key not in _cache:
        _cache[key] = _build(n_rows, apply_mask, trivial_lno)
    return _cache[key]


def kernel(x, context, mask, ln1_s, ln1_b, lnc_s, lnc_b, Wq, Wkv, null_kv, Wo,
           lno_s, lno_b, _n_rows=None, _return_bkr=False, _trace=False):
    x = np.asarray(x); context = np.asarray(context); mask = np.asarray(mask)
    n_rows = _n_rows or x.shape[1]
    Wq = np.asarray(Wq, np.float32); Wkv = np.asarray(Wkv, np.float32)
    Wo = np.asarray(Wo, np.float32); null_kv = np.asarray(null_kv, np.float32)
    ln1_s = np.asarray(ln1_s, np.float32); ln1_b = np.asarray(ln1_b, np.float32)
    lnc_s = np.asarray(lnc_s, np.float32); lnc_b = np.asarray(lnc_b, np.float32)
    lno_s = np.asarray(lno_s, np.float32); lno_b = np.asarray(lno_b, np.float32)

    Wk, Wv = Wkv[:, :INNER], Wkv[:, INNER:]
    wq_eff = (ln1_s[:, None] * Wq * SCALE).astype(NPBF)
    wk_eff = (lnc_s[:, None] * Wk).astype(NPBF)
    wv_eff = (lnc_s[:, None] * Wv).astype(NPBF)
    bq = (ln1_b @ Wq) * SCALE
    bk = ln1_b @ Wk * 0 + lnc_b @ Wk
    bv = lnc_b @ Wv
    assert np.abs(bq).max() == 0 and np.abs(bk).max() == 0 and np.abs(bv).max() == 0, \
        "nonzero LN biases not supported by this build"
    wo_bf = Wo.astype(NPBF)
    wmean = (-(Wo @ np.ones((DIM, 1), np.float32)) / DIM).astype(NPBF)
    nullkblk = np.zeros((128, 8, 16), np.float32)
    for j in range(8):
        nullkblk[0:64, j, 2 * j] = null_kv[0]
        nullkblk[64:128, j, 2 * j + 1] = null_kv[0]
    nullkblk = nullkblk.astype(NPBF)
    nullv1 = np.concatenate([null_kv[1], [1.0]]).reshape(1, 65).astype(NPBF)

    trivial_lno = bool(np.all(lno_s == 1.0) and np.all(lno_b == 0.0))
    apply_mask = not bool(mask.all())
    nc = _get_nc(n_rows, apply_mask, trivial_lno)

    in_maps = []
    for core in range(B):
        mc = np.ones((128, 2), np.float32)
        if apply_mask:
            mc = mask[core].reshape(2, 128).T.astype(np.float32)
        in_maps.append({
            "x": x[core, :n_rows].astype(NPBF),
            "ctx": context[core].astype(NPBF),
            "wq": wq_eff, "wk": wk_eff, "wv": wv_eff, "wo": wo_bf,
            "wmean": wmean, "nullkblk": nullkblk, "nullv1": nullv1,
            "maskcol": mc.astype(NPBF),
            "lnos": lno_s.reshape(1, DIM), "lnob": lno_b.reshape(1, DIM),
        })
    bkr = run_bass_kernel_spmd(nc, in_maps, core_ids=list(range(B)), trace=_trace)
    out = np.stack([bkr.results[core]["out"] for core in range(B)])
    if _return_bkr:
        return out, bkr
    return out



# revision 16
# speedup vs baseline: 1.6411x; 1.6411x over previous
"""Cross-attention Trainium2 kernel, batch-parallel across 8 NeuronCores.

Per core: one batch element. LN(x) -> qT via transposed projection,
LN(ctx) -> kT / v, transposed-layout attention (keys on partitions).
Softmax normalization is batched per chunk: the per-head partition-64
ones-row of the attn*V accumulation gives Z; Z rows are gathered to a
[16,512] tile via tiny PSUM->SBUF DMAs, 1/Z = exp(-ln Z) on the scalar
engine (activation table pinned to the ln+exp set so there are no
ACT_TABLE_LOADs), and the reciprocal is broadcast to 64 partitions with
one DRAM round-trip per chunk. Sim matmuls are row-tiled two heads at a
time (contract dim 64 -> PE array halves run concurrently). All matmuls
bf16 with f32 PSUM accumulation. LN scale/bias and the num_heads**-0.5
factor are folded into the weights on the host.
"""
import numpy as np
import ml_dtypes

import concourse.bass as bass
from concourse import bacc
import concourse.mybir as mybir
import concourse.tile as tile
from concourse.bass_utils import run_bass_kernel_spmd
from concourse.masks import make_identity

BF = mybir.dt.bfloat16
F32 = mybir.dt.float32
NPBF = ml_dtypes.bfloat16

B, N_FULL, M, DIM = 8, 4096, 256, 1024
H, D = 16, 64
INNER = H * D
EPS = 1e-6
SCALE = H ** -0.5

_cache = {}


def _ln_exp_table_id():
    """Index of the activation-function set containing both ln and exp.
    Falls back to the known trn2 index if the table file can't be read."""
    try:
        from concourse.hw_specs import get_activation_tables
        tabs = get_activation_tables("Tonga4")
        for i, (name, s) in enumerate(tabs.items()):
            names = {x.name for x in s}
            if "Ln" in names and "Exp" in names:
                return i
    except Exception:
        pass
    return 6


def _build(n_rows, apply_mask, trivial_lno):
    nchunks = n_rows // 512
    nc = bacc.Bacc(None, target_bir_lowering=False)
    x_d = nc.dram_tensor("x", [n_rows, DIM], BF, kind="ExternalInput")
    ctx_d = nc.dram_tensor("ctx", [M, DIM], BF, kind="ExternalInput")
    wq_d = nc.dram_tensor("wq", [DIM, INNER], BF, kind="ExternalInput")
    wk_d = nc.dram_tensor("wk", [DIM, INNER], BF, kind="ExternalInput")
    wv_d = nc.dram_tensor("wv", [DIM, INNER], BF, kind="ExternalInput")
    wo_d = nc.dram_tensor("wo", [INNER, DIM], BF, kind="ExternalInput")
    wmean_d = nc.dram_tensor("wmean", [INNER, 1], BF, kind="ExternalInput")  # -Wo@1/DIM
    nullkblk_d = nc.dram_tensor("nullkblk", [128, 8, 16], BF, kind="ExternalInput")
    nullv4_d = nc.dram_tensor("nullv4", [128, 65], BF, kind="ExternalInput")
    maskcol_d = nc.dram_tensor("maskcol", [128, 2], BF, kind="ExternalInput")
    lnos_d = nc.dram_tensor("lnos", [1, DIM], F32, kind="ExternalInput")
    lnob_d = nc.dram_tensor("lnob", [1, DIM], F32, kind="ExternalInput")
    out_d = nc.dram_tensor("out", [n_rows, DIM], BF, kind="ExternalOutput")

    with tile.TileContext(nc) as tc:
        with tc.tile_pool(name="const", bufs=1) as cst, \
             tc.tile_pool(name="sbw", bufs=1) as sbw, \
             tc.tile_pool(name="sbx", bufs=2) as sbx, \
             tc.tile_pool(name="sbq", bufs=2) as sbq, \
             tc.tile_pool(name="sba", bufs=2) as sba, \
             tc.tile_pool(name="sbo", bufs=2) as sbo, \
             tc.tile_pool(name="pproj", bufs=2, space="PSUM") as pproj, \
             tc.tile_pool(name="ptr", bufs=2, space="PSUM") as ptrp, \
             tc.tile_pool(name="psim", bufs=2, space="PSUM") as psim, \
             tc.tile_pool(name="pout", bufs=2, space="PSUM") as pout, \
             tc.tile_pool(name="drp", bufs=2, space="DRAM") as drp:

            # Pin the scalar-engine activation table to the set containing
            # ln+exp+square+copy so the compiler's greedy per-function table
            # chooser never inserts an ACT_TABLE_LOAD (1.28us each).
            nc.scalar.add_instruction(mybir.InstLoadActFuncSet(
                name=nc.get_next_instruction_name(),
                act_func_set_id=_ln_exp_table_id(), ins=[], outs=[]))

            ident = cst.tile([128, 128], BF, tag="ident")
            make_identity(nc, ident)
            epst = cst.tile([128, 1], F32, tag="epst")
            nc.vector.memset(epst, EPS)
            nullv4 = cst.tile([128, 65], BF, tag="nullv4")
            nc.sync.dma_start(out=nullv4, in_=nullv4_d[:, :])
            nullkblk = cst.tile([128, 8, 16], BF, tag="nullkblk")
            nc.sync.dma_start(out=nullkblk, in_=nullkblk_d[:, :, :])
            wmean = cst.tile([128, 8, 1], BF, tag="wmean")
            nc.sync.dma_start(out=wmean, in_=wmean_d.rearrange("(j p) o -> p j o", p=128))
            if apply_mask:
                maskcol = cst.tile([128, 2], BF, tag="maskcol")
                nc.sync.dma_start(out=maskcol, in_=maskcol_d[:, :])
            if not trivial_lno:
                lnos = cst.tile([128, DIM], F32, tag="lnos")
                lnob = cst.tile([128, DIM], F32, tag="lnob")
                nc.sync.dma_start(out=lnos, in_=bass.AP(
                    tensor=lnos_d, offset=0, ap=[[0, 128], [1, DIM]]))
                nc.sync.dma_start(out=lnob, in_=bass.AP(
                    tensor=lnob_d, offset=0, ap=[[0, 128], [1, DIM]]))

            # weights: [128, j, ...] partition-tiled over contraction dim.
            # wk/wv are only read during the context phase; they borrow the
            # S_sb rotation slots (same 16KB/partition) so their space is
            # recycled for the per-chunk attention numerators afterwards.
            wq = sbw.tile([128, 8, INNER], BF, tag="wq")
            wk = sbo.tile([128, 8, INNER], BF, tag="S_sb")
            wv = sbo.tile([128, 8, INNER], BF, tag="S_sb")
            wo = sbw.tile([128, 8, DIM], BF, tag="wo")
            for j in range(8):
                nc.sync.dma_start(out=wq[:, j], in_=wq_d.rearrange("(j p) i -> p j i", p=128)[:, j])
                nc.sync.dma_start(out=wk[:, j], in_=wk_d.rearrange("(j p) i -> p j i", p=128)[:, j])
                nc.sync.dma_start(out=wv[:, j], in_=wv_d.rearrange("(j p) i -> p j i", p=128)[:, j])
                nc.sync.dma_start(out=wo[:, j], in_=wo_d.rearrange("(j p) i -> p j i", p=128)[:, j])

            def rstd_of(var_ap, dst, tmp_pool, scale=1.0):
                """dst = (scale*var + eps)^-0.5 via Ln+Exp (pinned table)."""
                nc.scalar.activation(dst, var_ap, mybir.ActivationFunctionType.Ln,
                                     bias=epst, scale=scale)
                nc.scalar.activation(dst, dst, mybir.ActivationFunctionType.Exp,
                                     scale=-0.5)

            def layernorm_rows(dst_bf, src_tile, tmp_pool):
                """LN rows of [128, DIM] src -> bf16 dst."""
                stats = tmp_pool.tile([128, 2, 6], F32, tag="stats")
                nc.vector.bn_stats(stats[:, 0, :], src_tile[:, 0:512])
                nc.vector.bn_stats(stats[:, 1, :], src_tile[:, 512:1024])
                mv = tmp_pool.tile([128, 2], F32, tag="mv")
                nc.vector.bn_aggr(mv, stats)
                rstd = tmp_pool.tile([128, 1], F32, tag="rstd")
                rstd_of(mv[:, 1:2], rstd, tmp_pool)
                nc.vector.tensor_scalar(out=dst_bf, in0=src_tile,
                                        scalar1=mv[:, 0:1], scalar2=rstd,
                                        op0=mybir.AluOpType.subtract,
                                        op1=mybir.AluOpType.mult)

            # ---------------- context phase ----------------
            cnT = sbw.tile([128, 8, 256], BF, tag="cnT")
            for mm in range(2):
                ctile = sbx.tile([128, DIM], BF, tag="ctile")
                nc.sync.dma_start(out=ctile, in_=ctx_d[128 * mm:128 * (mm + 1), :])
                cn = sbx.tile([128, DIM], BF, tag="cn")
                layernorm_rows(cn, ctile, sbx)
                for g in range(2):
                    ptr = ptrp.tile([128, 512], BF, tag="ptr")
                    for b4 in range(4):
                        jj = g * 4 + b4
                        nc.tensor.transpose(ptr[:, 128 * b4:128 * (b4 + 1)],
                                            cn[:, 128 * jj:128 * (jj + 1)], ident)
                    nc.vector.tensor_copy(
                        cnT[:, g * 4:(g + 1) * 4, 128 * mm:128 * (mm + 1)],
                        ptr.rearrange("p (a b) -> p a b", a=4))

            kT = sbw.tile([128, 8, 256], BF, tag="kT")
            for i in range(8):
                pk = pproj.tile([128, 512], F32, tag="proj")
                for j in range(8):
                    nc.tensor.matmul(pk[:, 0:256], wk[:, j, 128 * i:128 * (i + 1)],
                                     cnT[:, j, :], start=(j == 0), stop=(j == 7))
                nc.vector.tensor_copy(kT[:, i, :], pk[:, 0:256])

            v_sb = sbw.tile([128, 2, 16, 65], BF, tag="v_sb")
            for mm in range(2):
                for nh in range(2):
                    pv = pproj.tile([128, 512], F32, tag="proj")
                    for j in range(8):
                        nc.tensor.matmul(pv, cnT[:, j, 128 * mm:128 * (mm + 1)],
                                         wv[:, j, 512 * nh:512 * (nh + 1)],
                                         start=(j == 0), stop=(j == 7))
                    nc.vector.tensor_copy(
                        v_sb[:, mm, 8 * nh:8 * (nh + 1), 0:64],
                        pv.rearrange("p (h d) -> p h d", h=8))
                nc.vector.memset(v_sb[:, mm, :, 64:65], 1.0)

            # ---------------- main loop over 512-row chunks ----------------
            for c in range(nchunks):
                # --- phase A: x LN + transpose + Q projection + null scores
                xnT = sbq.tile([128, 8, 512], BF, tag="xnT", bufs=1)
                for r in range(4):
                    xbf = sbx.tile([128, DIM], BF, tag="xbf")
                    nc.sync.dma_start(out=xbf, in_=x_d[c * 512 + 128 * r: c * 512 + 128 * (r + 1), :])
                    xn = sbx.tile([128, DIM], BF, tag="xn")
                    layernorm_rows(xn, xbf, sbx)
                    for g in range(2):
                        ptr = ptrp.tile([128, 512], BF, tag="ptr")
                        for b4 in range(4):
                            jj = g * 4 + b4
                            nc.tensor.transpose(ptr[:, 128 * b4:128 * (b4 + 1)],
                                                xn[:, 128 * jj:128 * (jj + 1)], ident)
                        nc.vector.tensor_copy(
                            xnT[:, g * 4:(g + 1) * 4, 128 * r:128 * (r + 1)],
                            ptr.rearrange("p (a b) -> p a b", a=4))

                qT = sbq.tile([128, 8, 512], BF, tag="qT")
                for i in range(8):
                    pq = pproj.tile([128, 512], F32, tag="proj")
                    for j in range(8):
                        nc.tensor.matmul(pq, wq[:, j, 128 * i:128 * (i + 1)], xnT[:, j, :],
                                         start=(j == 0), stop=(j == 7))
                    # PSUM f32 -> SBUF bf16 copy on the scalar engine (Copy is
                    # in the pinned table; keeps DVE free)
                    nc.scalar.activation(qT[:, i, :], pq,
                                         mybir.ActivationFunctionType.Copy)

                # null-key scores for all heads: [16, 512]
                pnull = pproj.tile([16, 512], F32, tag="proj")
                for j in range(8):
                    nc.tensor.matmul(pnull, nullkblk[:, j, :], qT[:, j, :],
                                     start=(j == 0), stop=(j == 7))
                # enull16 partition p holds head 4*(p%4)+p//4 (nullkblk columns
                # are permuted on the host); the DMA spreads the 16 rows to
                # partitions {0,32,64,96} x 4 slots so the rank-1 null-value
                # matmuls can be row-tiled.
                enull16 = sba.tile([16, 512], BF, tag="enull16")
                nc.scalar.activation(enull16, pnull, mybir.ActivationFunctionType.Exp)
                enullf = sba.tile([97, 4, 512], BF, tag="enullf")
                for k in range(4):
                    nc.sync.dma_start(out=enullf[32 * k:32 * k + 1, :, :],
                                      in_=enull16[4 * k:4 * k + 4, :])

                # --- phases B+C: per-head sim (row-tiled pairs) + attn*V
                S_sb = sbo.tile([65, 16, 512], BF, tag="S_sb")
                for h in range(H):
                    j, po = h // 2, 64 * (h % 2)
                    ps0 = psim.tile([128, 512], F32, tag="sim")
                    ps1 = psim.tile([128, 512], F32, tag="sim")
                    nc.tensor.matmul(ps0, kT[po:po + 64, j, 0:128], qT[po:po + 64, j, :],
                                     start=True, stop=True, tile_position=(po, 0))
                    nc.tensor.matmul(ps1, kT[po:po + 64, j, 128:256], qT[po:po + 64, j, :],
                                     start=True, stop=True, tile_position=(po, 0))
                    eT = sba.tile([128, 2, 512], BF, tag="eT", bufs=4)
                    nc.scalar.activation(eT[:, 0, :], ps0, mybir.ActivationFunctionType.Exp)
                    nc.scalar.activation(eT[:, 1, :], ps1, mybir.ActivationFunctionType.Exp)
                    if apply_mask:
                        nc.vector.tensor_scalar_mul(eT[:, 0, :], in0=eT[:, 0, :],
                                                    scalar1=maskcol[:, 0:1])
                        nc.vector.tensor_scalar_mul(eT[:, 1, :], in0=eT[:, 1, :],
                                                    scalar1=maskcol[:, 1:2])
                    po_ps = pout.tile([65, 512], F32, tag="out")
                    np4 = 32 * (h % 4)
                    nc.tensor.matmul(po_ps, v_sb[:, 0, h, :], eT[:, 0, :], start=True, stop=False)
                    nc.tensor.matmul(po_ps, v_sb[:, 1, h, :], eT[:, 1, :], start=False, stop=False)
                    nc.tensor.matmul(po_ps, nullv4[np4:np4 + 1, :],
                                     enullf[np4:np4 + 1, h // 4, :],
                                     start=False, stop=True, tile_position=(np4, 0))
                    # S (and the Z row at partition 64) -> SBUF bf16,
                    # alternating DVE/scalar to balance engine load
                    if h % 2 == 0:
                        nc.vector.tensor_copy(S_sb[:, h, :], po_ps)
                    else:
                        nc.scalar.activation(S_sb[:, h, :], po_ps,
                                             mybir.ActivationFunctionType.Copy)

                # --- phase D: rec = 1/Z via exp(-ln Z); broadcast via DRAM
                # repartition the 16 Z rows (all on partition 64) to [16, 512]
                Zt = sba.tile([16, 512], BF, tag="Zt", bufs=1)
                nc.sync.dma_start(out=Zt, in_=S_sb[64:65, :, :])
                lnz = sba.tile([16, 512], F32, tag="lnz", bufs=1)
                nc.scalar.activation(lnz, Zt, mybir.ActivationFunctionType.Ln)
                rec16 = sba.tile([16, 512], BF, tag="rec16", bufs=1)
                nc.scalar.activation(rec16, lnz, mybir.ActivationFunctionType.Exp,
                                     scale=-1.0)
                rc_d = drp.tile([16, 512], BF, tag="rc_d")
                nc.sync.dma_start(out=rc_d[:, :], in_=rec16)
                recb = sbo.tile([64, 16, 512], BF, tag="recb", bufs=1)
                nc.sync.dma_start(out=recb, in_=bass.AP(
                    tensor=rc_d.tensor, offset=rc_d.offset,
                    ap=[[0, 64], [512, 16], [1, 512]]))

                # --- phase E: outT = S * rec
                outT = sbo.tile([128, 8, 512], BF, tag="outT")
                for h in range(H):
                    j, po = h // 2, 64 * (h % 2)
                    nc.vector.tensor_mul(outT[po:po + 64, j, :], S_sb[0:64, h, :],
                                         recb[:, h, :])

                # --- phase F: out projection + final LN (row space)
                for m in range(4):
                    pmean = pout.tile([128, 1], F32, tag="out")
                    for j in range(8):
                        nc.tensor.matmul(pmean, outT[:, j, 128 * m:128 * (m + 1)],
                                         wmean[:, j, :], start=(j == 0), stop=(j == 7))
                    negmu = sbx.tile([128, 1], F32, tag="negmu")
                    nc.vector.tensor_copy(negmu, pmean)
                    fins = []
                    sumsqs = []
                    for nh in range(2):
                        pf = pproj.tile([128, 512], F32, tag="proj")
                        for j in range(8):
                            nc.tensor.matmul(pf, outT[:, j, 128 * m:128 * (m + 1)],
                                             wo[:, j, 512 * nh:512 * (nh + 1)],
                                             start=(j == 0), stop=(j == 7))
                        junk = sbx.tile([128, 512], BF, tag="junk")
                        ssq = sbx.tile([128, 1], F32, tag=f"ssq{nh}")
                        nc.scalar.activation(junk, pf, mybir.ActivationFunctionType.Square,
                                             bias=negmu, scale=1.0, accum_out=ssq)
                        fins.append(pf)
                        sumsqs.append(ssq)
                    var = sbx.tile([128, 1], F32, tag="var")
                    nc.vector.tensor_add(var, sumsqs[0], sumsqs[1])
                    rstd_o = sbx.tile([128, 1], F32, tag="rstd_o")
                    rstd_of(var, rstd_o, sbx, scale=1.0 / DIM)
                    orow = sbo.tile([128, DIM], BF, tag="orow")
                    for nh in range(2):
                        nc.vector.tensor_scalar(out=orow[:, 512 * nh:512 * (nh + 1)],
                                                in0=fins[nh], scalar1=negmu, scalar2=rstd_o,
                                                op0=mybir.AluOpType.add,
                                                op1=mybir.AluOpType.mult)
                    if not trivial_lno:
                        nc.vector.tensor_mul(orow, orow, lnos)
                        nc.vector.tensor_add(orow, orow, lnob)
                    nc.sync.dma_start(out=out_d[c * 512 + 128 * m: c * 512 + 128 * (m + 1), :],
                                      in_=orow)
    nc.compile()
    return nc


def _get_nc(n_rows, apply_mask, trivial_lno):
    key = (n_rows, apply_mask, trivial_lno)
    if key not in _cache:
        _cache[key] = _build(n_rows, apply_mask, trivial_lno)
    return _cache[key]


def kernel(x, context, mask, ln1_s, ln1_b, lnc_s, lnc_b, Wq, Wkv, null_kv, Wo,
           lno_s, lno_b, _n_rows=None, _return_bkr=False, _trace=False):
    x = np.asarray(x); context = np.asarray(context); mask = np.asarray(mask)
    n_rows = _n_rows or x.shape[1]
    Wq = np.asarray(Wq, np.float32); Wkv = np.asarray(Wkv, np.float32)
    Wo = np.asarray(Wo, np.float32); null_kv = np.asarray(null_kv, np.float32)
    ln1_s = np.asarray(ln1_s, np.float32); ln1_b = np.asarray(ln1_b, np.float32)
    lnc_s = np.asarray(lnc_s, np.float32); lnc_b = np.asarray(lnc_b, np.float32)
    lno_s = np.asarray(lno_s, np.float32); lno_b = np.asarray(lno_b, np.float32)

    Wk, Wv = Wkv[:, :INNER], Wkv[:, INNER:]
    wq_eff = (ln1_s[:, None] * Wq * SCALE).astype(NPBF)
    wk_eff = (lnc_s[:, None] * Wk).astype(NPBF)
    wv_eff = (lnc_s[:, None] * Wv).astype(NPBF)
    bq = (ln1_b @ Wq) * SCALE
    bk = lnc_b @ Wk
    bv = lnc_b @ Wv
    assert np.abs(bq).max() == 0 and np.abs(bk).max() == 0 and np.abs(bv).max() == 0, \
        "nonzero LN biases not supported by this build"
    wo_bf = Wo.astype(NPBF)
    wmean = (-(Wo @ np.ones((DIM, 1), np.float32)) / DIM).astype(NPBF)
    # head h's null score lands at pnull partition 4*(h%4)+h//4 so the
    # enull spread-DMA puts head h at partition 32*(h%4), slot h//4
    nullkblk = np.zeros((128, 8, 16), np.float32)
    for h in range(16):
        j = h // 2
        rows = slice(0, 64) if h % 2 == 0 else slice(64, 128)
        nullkblk[rows, j, 4 * (h % 4) + h // 4] = null_kv[0]
    nullkblk = nullkblk.astype(NPBF)
    nullv4 = np.zeros((128, 65), np.float32)
    for k in range(4):
        nullv4[32 * k] = np.concatenate([null_kv[1], [1.0]])
    nullv4 = nullv4.astype(NPBF)

    trivial_lno = bool(np.all(lno_s == 1.0) and np.all(lno_b == 0.0))
    apply_mask = not bool(mask.all())
    nc = _get_nc(n_rows, apply_mask, trivial_lno)

    in_maps = []
    for core in range(B):
        mc = np.ones((128, 2), np.float32)
        if apply_mask:
            mc = mask[core].reshape(2, 128).T.astype(np.float32)
        in_maps.append({
            "x": x[core, :n_rows].astype(NPBF),
            "ctx": context[core].astype(NPBF),
            "wq": wq_eff, "wk": wk_eff, "wv": wv_eff, "wo": wo_bf,
            "wmean": wmean, "nullkblk": nullkblk, "nullv4": nullv4,
            "maskcol": mc.astype(NPBF),
            "lnos": lno_s.reshape(1, DIM), "lnob": lno_b.reshape(1, DIM),
        })
    bkr = run_bass_kernel_spmd(nc, in_maps, core_ids=list(range(B)), trace=_trace)
    out = np.stack([bkr.results[core]["out"].astype(np.float32) for core in range(B)])
    if _return_bkr:
        return out, bkr
    return out


# revision 19
# speedup vs baseline: 2.0817x; 1.2685x over previous
"""Cross-attention Trainium2 kernel, batch-parallel across 8 NeuronCores.

Per core: one batch element. LN(x) -> qT via transposed projection,
LN(ctx) -> kT / v, transposed-layout attention (keys on partitions).
Softmax normalization is batched per chunk: the per-head partition-64
ones-row of the attn*V accumulation gives Z; Z rows are gathered to a
[16,512] tile via tiny PSUM->SBUF DMAs, 1/Z = exp(-ln Z) on the scalar
engine (activation table pinned to the ln+exp set so there are no
ACT_TABLE_LOADs), and the reciprocal is broadcast to 64 partitions with
one DRAM round-trip per chunk. Sim matmuls are row-tiled two heads at a
time (contract dim 64 -> PE array halves run concurrently). All matmuls
bf16 with f32 PSUM accumulation. LN scale/bias and the num_heads**-0.5
factor are folded into the weights on the host.
"""
import numpy as np
import ml_dtypes

import concourse.bass as bass
from concourse import bacc
import concourse.mybir as mybir
import concourse.tile as tile
from concourse.bass_utils import run_bass_kernel_spmd
from concourse.masks import make_identity

BF = mybir.dt.bfloat16
F32 = mybir.dt.float32
NPBF = ml_dtypes.bfloat16

B, N_FULL, M, DIM = 8, 4096, 256, 1024
H, D = 16, 64
INNER = H * D
EPS = 1e-6
SCALE = H ** -0.5

_cache = {}


def _ln_exp_table_id():
    """Index of the activation-function set containing both ln and exp.
    Falls back to the known trn2 index if the table file can't be read."""
    try:
        from concourse.hw_specs import get_activation_tables
        tabs = get_activation_tables("Tonga4")
        for i, (name, s) in enumerate(tabs.items()):
            names = {x.name for x in s}
            if "Ln" in names and "Exp" in names:
                return i
    except Exception:
        pass
    return 6


def _build(n_rows, apply_mask, trivial_lno):
    nchunks = n_rows // 512
    nc = bacc.Bacc(None, target_bir_lowering=False)
    x_d = nc.dram_tensor("x", [n_rows, DIM], BF, kind="ExternalInput")
    ctx_d = nc.dram_tensor("ctx", [M, DIM], BF, kind="ExternalInput")
    wq_d = nc.dram_tensor("wq", [DIM, INNER], BF, kind="ExternalInput")
    wk_d = nc.dram_tensor("wk", [DIM, INNER], BF, kind="ExternalInput")
    wv_d = nc.dram_tensor("wv", [DIM, INNER], BF, kind="ExternalInput")
    wo_d = nc.dram_tensor("wo", [INNER, DIM], BF, kind="ExternalInput")
    wmean_d = nc.dram_tensor("wmean", [INNER, 1], BF, kind="ExternalInput")  # -Wo@1/DIM
    nullkblk_d = nc.dram_tensor("nullkblk", [128, 8, 16], BF, kind="ExternalInput")
    nullv4_d = nc.dram_tensor("nullv4", [128, 65], BF, kind="ExternalInput")
    maskcol_d = nc.dram_tensor("maskcol", [128, 2], BF, kind="ExternalInput")
    lnos_d = nc.dram_tensor("lnos", [1, DIM], F32, kind="ExternalInput")
    lnob_d = nc.dram_tensor("lnob", [1, DIM], F32, kind="ExternalInput")
    out_d = nc.dram_tensor("out", [n_rows, DIM], BF, kind="ExternalOutput")

    with tile.TileContext(nc) as tc:
        with tc.tile_pool(name="const", bufs=1) as cst, \
             tc.tile_pool(name="sbw", bufs=1) as sbw, \
             tc.tile_pool(name="sbx", bufs=2) as sbx, \
             tc.tile_pool(name="sbq", bufs=2) as sbq, \
             tc.tile_pool(name="sba", bufs=2) as sba, \
             tc.tile_pool(name="sbo", bufs=2) as sbo, \
             tc.tile_pool(name="pproj", bufs=2, space="PSUM") as pproj, \
             tc.tile_pool(name="ptr", bufs=2, space="PSUM") as ptrp, \
             tc.tile_pool(name="psim", bufs=2, space="PSUM") as psim, \
             tc.tile_pool(name="pout", bufs=2, space="PSUM") as pout, \
             tc.tile_pool(name="drp", bufs=2, space="DRAM") as drp:

            # Pin the scalar-engine activation table to the set containing
            # ln+exp+square+copy so the compiler's greedy per-function table
            # chooser never inserts an ACT_TABLE_LOAD (1.28us each).
            nc.scalar.add_instruction(mybir.InstLoadActFuncSet(
                name=nc.get_next_instruction_name(),
                act_func_set_id=_ln_exp_table_id(), ins=[], outs=[]))

            ident = cst.tile([128, 128], BF, tag="ident")
            make_identity(nc, ident)
            epst = cst.tile([128, 1], F32, tag="epst")
            nc.vector.memset(epst, EPS)
            nullv4 = cst.tile([128, 65], BF, tag="nullv4")
            nc.sync.dma_start(out=nullv4, in_=nullv4_d[:, :])
            nullkblk = cst.tile([128, 8, 16], BF, tag="nullkblk")
            nc.sync.dma_start(out=nullkblk, in_=nullkblk_d[:, :, :])
            wmean = cst.tile([128, 8, 1], BF, tag="wmean")
            nc.sync.dma_start(out=wmean, in_=wmean_d.rearrange("(j p) o -> p j o", p=128))
            if apply_mask:
                maskcol = cst.tile([128, 2], BF, tag="maskcol")
                nc.sync.dma_start(out=maskcol, in_=maskcol_d[:, :])
            if not trivial_lno:
                lnos = cst.tile([128, DIM], F32, tag="lnos")
                lnob = cst.tile([128, DIM], F32, tag="lnob")
                nc.sync.dma_start(out=lnos, in_=bass.AP(
                    tensor=lnos_d, offset=0, ap=[[0, 128], [1, DIM]]))
                nc.sync.dma_start(out=lnob, in_=bass.AP(
                    tensor=lnob_d, offset=0, ap=[[0, 128], [1, DIM]]))

            # weights: [128, j, ...] partition-tiled over contraction dim.
            # wk/wv are only read during the context phase; they borrow the
            # S_sb rotation slots (same 16KB/partition) so their space is
            # recycled for the per-chunk attention numerators afterwards.
            wq = sbw.tile([128, 8, INNER], BF, tag="wq")
            wk = sbo.tile([128, 8, INNER], BF, tag="S_sb")
            wv = sbo.tile([128, 8, INNER], BF, tag="S_sb")
            wo = sbw.tile([128, 8, DIM], BF, tag="wo")
            for j in range(8):
                nc.sync.dma_start(out=wq[:, j], in_=wq_d.rearrange("(j p) i -> p j i", p=128)[:, j])
                nc.sync.dma_start(out=wk[:, j], in_=wk_d.rearrange("(j p) i -> p j i", p=128)[:, j])
                nc.sync.dma_start(out=wv[:, j], in_=wv_d.rearrange("(j p) i -> p j i", p=128)[:, j])
                nc.sync.dma_start(out=wo[:, j], in_=wo_d.rearrange("(j p) i -> p j i", p=128)[:, j])

            def rstd_of(var_ap, dst, tmp_pool, scale=1.0):
                """dst = (scale*var + eps)^-0.5 via Ln+Exp (pinned table)."""
                nc.scalar.activation(dst, var_ap, mybir.ActivationFunctionType.Ln,
                                     bias=epst, scale=scale)
                nc.scalar.activation(dst, dst, mybir.ActivationFunctionType.Exp,
                                     scale=-0.5)

            def layernorm_rows(dst_bf, src_tile, tmp_pool):
                """LN rows of [128, DIM] src -> bf16 dst."""
                stats = tmp_pool.tile([128, 2, 6], F32, tag="stats")
                nc.vector.bn_stats(stats[:, 0, :], src_tile[:, 0:512])
                nc.vector.bn_stats(stats[:, 1, :], src_tile[:, 512:1024])
                mv = tmp_pool.tile([128, 2], F32, tag="mv")
                nc.vector.bn_aggr(mv, stats)
                rstd = tmp_pool.tile([128, 1], F32, tag="rstd")
                rstd_of(mv[:, 1:2], rstd, tmp_pool)
                nc.vector.tensor_scalar(out=dst_bf, in0=src_tile,
                                        scalar1=mv[:, 0:1], scalar2=rstd,
                                        op0=mybir.AluOpType.subtract,
                                        op1=mybir.AluOpType.mult)

            # ---------------- context phase ----------------
            cnT = sbw.tile([128, 8, 256], BF, tag="cnT")
            for mm in range(2):
                ctile = sbx.tile([128, DIM], BF, tag="ctile")
                nc.sync.dma_start(out=ctile, in_=ctx_d[128 * mm:128 * (mm + 1), :])
                cn = sbx.tile([128, DIM], BF, tag="cn")
                layernorm_rows(cn, ctile, sbx)
                for g in range(2):
                    ptr = ptrp.tile([128, 512], BF, tag="ptr")
                    for b4 in range(4):
                        jj = g * 4 + b4
                        nc.tensor.transpose(ptr[:, 128 * b4:128 * (b4 + 1)],
                                            cn[:, 128 * jj:128 * (jj + 1)], ident)
                    nc.vector.tensor_copy(
                        cnT[:, g * 4:(g + 1) * 4, 128 * mm:128 * (mm + 1)],
                        ptr.rearrange("p (a b) -> p a b", a=4))

            kT = sbw.tile([128, 8, 256], BF, tag="kT")
            for i in range(8):
                pk = pproj.tile([128, 512], F32, tag="proj")
                for j in range(8):
                    nc.tensor.matmul(pk[:, 0:256], wk[:, j, 128 * i:128 * (i + 1)],
                                     cnT[:, j, :], start=(j == 0), stop=(j == 7))
                nc.vector.tensor_copy(kT[:, i, :], pk[:, 0:256])

            v_sb = sbw.tile([128, 2, 16, 65], BF, tag="v_sb")
            for mm in range(2):
                for nh in range(2):
                    pv = pproj.tile([128, 512], F32, tag="proj")
                    for j in range(8):
                        nc.tensor.matmul(pv, cnT[:, j, 128 * mm:128 * (mm + 1)],
                                         wv[:, j, 512 * nh:512 * (nh + 1)],
                                         start=(j == 0), stop=(j == 7))
                    nc.vector.tensor_copy(
                        v_sb[:, mm, 8 * nh:8 * (nh + 1), 0:64],
                        pv.rearrange("p (h d) -> p h d", h=8))
                nc.vector.memset(v_sb[:, mm, :, 64:65], 1.0)

            # ---------------- main loop over 512-row chunks ----------------
            # Software-pipelined: phases A-D of chunk c are issued BEFORE
            # phases E/F of chunk c-1, so the PE queue always holds
            # independent matmuls while chunk c-1's softmax-normalize tail
            # (Zt DMA -> ln -> exp -> broadcast DMA) resolves. Without this
            # the out-projection matmuls head-of-line-block the PE for
            # ~15us per chunk and the HAM clock gate re-throttles.

            def phase_front(c):
                # --- phase A: x LN + transpose + Q projection + null scores
                xnT = sbq.tile([128, 8, 512], BF, tag="xnT", bufs=1)
                for r in range(4):
                    xbf = sbx.tile([128, DIM], BF, tag="xbf")
                    nc.sync.dma_start(out=xbf, in_=x_d[c * 512 + 128 * r: c * 512 + 128 * (r + 1), :])
                    xn = sbx.tile([128, DIM], BF, tag="xn")
                    layernorm_rows(xn, xbf, sbx)
                    for g in range(2):
                        ptr = ptrp.tile([128, 512], BF, tag="ptr")
                        for b4 in range(4):
                            jj = g * 4 + b4
                            nc.tensor.transpose(ptr[:, 128 * b4:128 * (b4 + 1)],
                                                xn[:, 128 * jj:128 * (jj + 1)], ident)
                        nc.vector.tensor_copy(
                            xnT[:, g * 4:(g + 1) * 4, 128 * r:128 * (r + 1)],
                            ptr.rearrange("p (a b) -> p a b", a=4))

                qT = sbq.tile([128, 8, 512], BF, tag="qT")
                for i in range(8):
                    pq = pproj.tile([128, 512], F32, tag="proj")
                    for j in range(8):
                        nc.tensor.matmul(pq, wq[:, j, 128 * i:128 * (i + 1)], xnT[:, j, :],
                                         start=(j == 0), stop=(j == 7))
                    # PSUM f32 -> SBUF bf16 copy on the scalar engine (Copy is
                    # in the pinned table; keeps DVE free)
                    nc.scalar.activation(qT[:, i, :], pq,
                                         mybir.ActivationFunctionType.Copy)

                # null-key scores for all heads: [16, 512]
                pnull = pproj.tile([16, 512], F32, tag="proj")
                for j in range(8):
                    nc.tensor.matmul(pnull, nullkblk[:, j, :], qT[:, j, :],
                                     start=(j == 0), stop=(j == 7))
                # enull16 partition p holds head 4*(p%4)+p//4 (nullkblk columns
                # are permuted on the host); the DMA spreads the 16 rows to
                # partitions {0,32,64,96} x 4 slots so the rank-1 null-value
                # matmuls can be row-tiled.
                enull16 = sba.tile([16, 512], BF, tag="enull16")
                nc.scalar.activation(enull16, pnull, mybir.ActivationFunctionType.Exp)
                enullf = sba.tile([97, 4, 512], BF, tag="enullf")
                for k in range(4):
                    nc.sync.dma_start(out=enullf[32 * k:32 * k + 1, :, :],
                                      in_=enull16[4 * k:4 * k + 4, :])

                # --- phases B+C: per-head sim (row-tiled pairs) + attn*V
                S_sb = sbo.tile([65, 16, 512], BF, tag="S_sb")
                for h in range(H):
                    j, po = h // 2, 64 * (h % 2)
                    ps0 = psim.tile([128, 512], F32, tag="sim")
                    ps1 = psim.tile([128, 512], F32, tag="sim")
                    nc.tensor.matmul(ps0, kT[po:po + 64, j, 0:128], qT[po:po + 64, j, :],
                                     start=True, stop=True, tile_position=(po, 0))
                    nc.tensor.matmul(ps1, kT[po:po + 64, j, 128:256], qT[po:po + 64, j, :],
                                     start=True, stop=True, tile_position=(po, 0))
                    eT = sba.tile([128, 2, 512], BF, tag="eT", bufs=4)
                    nc.scalar.activation(eT[:, 0, :], ps0, mybir.ActivationFunctionType.Exp)
                    nc.scalar.activation(eT[:, 1, :], ps1, mybir.ActivationFunctionType.Exp)
                    if apply_mask:
                        nc.vector.tensor_scalar_mul(eT[:, 0, :], in0=eT[:, 0, :],
                                                    scalar1=maskcol[:, 0:1])
                        nc.vector.tensor_scalar_mul(eT[:, 1, :], in0=eT[:, 1, :],
                                                    scalar1=maskcol[:, 1:2])
                    po_ps = pout.tile([65, 512], F32, tag="out")
                    np4 = 32 * (h % 4)
                    nc.tensor.matmul(po_ps, v_sb[:, 0, h, :], eT[:, 0, :], start=True, stop=False)
                    nc.tensor.matmul(po_ps, v_sb[:, 1, h, :], eT[:, 1, :], start=False, stop=False)
                    nc.tensor.matmul(po_ps, nullv4[np4:np4 + 1, :],
                                     enullf[np4:np4 + 1, h // 4, :],
                                     start=False, stop=True, tile_position=(np4, 0))
                    # S (and the Z row at partition 64) -> SBUF bf16,
                    # alternating DVE/scalar to balance engine load
                    if h % 2 == 0:
                        nc.vector.tensor_copy(S_sb[:, h, :], po_ps)
                    else:
                        nc.scalar.activation(S_sb[:, h, :], po_ps,
                                             mybir.ActivationFunctionType.Copy)

                # --- phase D: rec = 1/Z via exp(-ln Z); broadcast via DRAM
                # repartition the 16 Z rows (all on partition 64) to [16, 512]
                Zt = sba.tile([16, 512], BF, tag="Zt", bufs=1)
                nc.sync.dma_start(out=Zt, in_=S_sb[64:65, :, :])
                lnz = sba.tile([16, 512], F32, tag="lnz", bufs=1)
                nc.scalar.activation(lnz, Zt, mybir.ActivationFunctionType.Ln)
                rec16 = sba.tile([16, 512], BF, tag="rec16", bufs=1)
                nc.scalar.activation(rec16, lnz, mybir.ActivationFunctionType.Exp,
                                     scale=-1.0)
                rc_d = drp.tile([16, 512], BF, tag="rc_d")
                nc.sync.dma_start(out=rc_d[:, :], in_=rec16)
                recb = sbo.tile([64, 16, 512], BF, tag="recb", bufs=2)
                nc.sync.dma_start(out=recb, in_=bass.AP(
                    tensor=rc_d.tensor, offset=rc_d.offset,
                    ap=[[0, 64], [512, 16], [1, 512]]))
                return S_sb, recb

            def phase_back(c, S_sb, recb):
                # --- phase E: outT = S * rec
                outT = sbo.tile([128, 8, 512], BF, tag="outT")
                for h in range(H):
                    j, po = h // 2, 64 * (h % 2)
                    nc.vector.tensor_mul(outT[po:po + 64, j, :], S_sb[0:64, h, :],
                                         recb[:, h, :])

                # --- phase F: out projection + final LN (row space)
                for m in range(4):
                    pmean = pout.tile([128, 1], F32, tag="out")
                    for j in range(8):
                        nc.tensor.matmul(pmean, outT[:, j, 128 * m:128 * (m + 1)],
                                         wmean[:, j, :], start=(j == 0), stop=(j == 7))
                    negmu = sbx.tile([128, 1], F32, tag="negmu")
                    nc.vector.tensor_copy(negmu, pmean)
                    fins = []
                    sumsqs = []
                    for nh in range(2):
                        pf = pproj.tile([128, 512], F32, tag="proj")
                        for j in range(8):
                            nc.tensor.matmul(pf, outT[:, j, 128 * m:128 * (m + 1)],
                                             wo[:, j, 512 * nh:512 * (nh + 1)],
                                             start=(j == 0), stop=(j == 7))
                        junk = sbx.tile([128, 512], BF, tag="junk")
                        ssq = sbx.tile([128, 1], F32, tag=f"ssq{nh}")
                        nc.scalar.activation(junk, pf, mybir.ActivationFunctionType.Square,
                                             bias=negmu, scale=1.0, accum_out=ssq)
                        fins.append(pf)
                        sumsqs.append(ssq)
                    var = sbx.tile([128, 1], F32, tag="var")
                    nc.vector.tensor_add(var, sumsqs[0], sumsqs[1])
                    rstd_o = sbx.tile([128, 1], F32, tag="rstd_o")
                    rstd_of(var, rstd_o, sbx, scale=1.0 / DIM)
                    orow = sbo.tile([128, DIM], BF, tag="orow")
                    for nh in range(2):
                        nc.vector.tensor_scalar(out=orow[:, 512 * nh:512 * (nh + 1)],
                                                in0=fins[nh], scalar1=negmu, scalar2=rstd_o,
                                                op0=mybir.AluOpType.add,
                                                op1=mybir.AluOpType.mult)
                    if not trivial_lno:
                        nc.vector.tensor_mul(orow, orow, lnos)
                        nc.vector.tensor_add(orow, orow, lnob)
                    nc.sync.dma_start(out=out_d[c * 512 + 128 * m: c * 512 + 128 * (m + 1), :],
                                      in_=orow)

            pend = None
            for it in range(nchunks + 1):
                cur = phase_front(it) if it < nchunks else None
                if pend is not None:
                    phase_back(it - 1, *pend)
                pend = cur
    nc.compile()
    return nc


def _get_nc(n_rows, apply_mask, trivial_lno):
    key = (n_rows, apply_mask, trivial_lno)
    if key not in _cache:
        _cache[key] = _build(n_rows, apply_mask, trivial_lno)
    return _cache[key]


def kernel(x, context, mask, ln1_s, ln1_b, lnc_s, lnc_b, Wq, Wkv, null_kv, Wo,
           lno_s, lno_b, _n_rows=None, _return_bkr=False, _trace=False):
    x = np.asarray(x); context = np.asarray(context); mask = np.asarray(mask)
    n_rows = _n_rows or x.shape[1]
    Wq = np.asarray(Wq, np.float32); Wkv = np.asarray(Wkv, np.float32)
    Wo = np.asarray(Wo, np.float32); null_kv = np.asarray(null_kv, np.float32)
    ln1_s = np.asarray(ln1_s, np.float32); ln1_b = np.asarray(ln1_b, np.float32)
    lnc_s = np.asarray(lnc_s, np.float32); lnc_b = np.asarray(lnc_b, np.float32)
    lno_s = np.asarray(lno_s, np.float32); lno_b = np.asarray(lno_b, np.float32)

    Wk, Wv = Wkv[:, :INNER], Wkv[:, INNER:]
    wq_eff = (ln1_s[:, None] * Wq * SCALE).astype(NPBF)
    wk_eff = (lnc_s[:, None] * Wk).astype(NPBF)
    wv_eff = (lnc_s[:, None] * Wv).astype(NPBF)
    bq = (ln1_b @ Wq) * SCALE
    bk = lnc_b @ Wk
    bv = lnc_b @ Wv
    assert np.abs(bq).max() == 0 and np.abs(bk).max() == 0 and np.abs(bv).max() == 0, \
        "nonzero LN biases not supported by this build"
    wo_bf = Wo.astype(NPBF)
    wmean = (-(Wo @ np.ones((DIM, 1), np.float32)) / DIM).astype(NPBF)
    # head h's null score lands at pnull partition 4*(h%4)+h//4 so the
    # enull spread-DMA puts head h at partition 32*(h%4), slot h//4
    nullkblk = np.zeros((128, 8, 16), np.float32)
    for h in range(16):
        j = h // 2
        rows = slice(0, 64) if h % 2 == 0 else slice(64, 128)
        nullkblk[rows, j, 4 * (h % 4) + h // 4] = null_kv[0]
    nullkblk = nullkblk.astype(NPBF)
    nullv4 = np.zeros((128, 65), np.float32)
    for k in range(4):
        nullv4[32 * k] = np.concatenate([null_kv[1], [1.0]])
    nullv4 = nullv4.astype(NPBF)

    trivial_lno = bool(np.all(lno_s == 1.0) and np.all(lno_b == 0.0))
    apply_mask = not bool(mask.all())
    nc = _get_nc(n_rows, apply_mask, trivial_lno)

    in_maps = []
    for core in range(B):
        mc = np.ones((128, 2), np.float32)
        if apply_mask:
            mc = mask[core].reshape(2, 128).T.astype(np.float32)
        in_maps.append({
            "x": x[core, :n_rows].astype(NPBF),
            "ctx": context[core].astype(NPBF),
            "wq": wq_eff, "wk": wk_eff, "wv": wv_eff, "wo": wo_bf,
            "wmean": wmean, "nullkblk": nullkblk, "nullv4": nullv4,
            "maskcol": mc.astype(NPBF),
            "lnos": lno_s.reshape(1, DIM), "lnob": lno_b.reshape(1, DIM),
        })
    bkr = run_bass_kernel_spmd(nc, in_maps, core_ids=list(range(B)), trace=_trace)
    out = np.stack([bkr.results[core]["out"].astype(np.float32) for core in range(B)])
    if _return_bkr:
        return out, bkr
    return out
